# revision 1
# baseline (speedup 1.0000x reference)
"""BrainRNN forward pass on 8 TRN2 NeuronCores (Bass/Tile, SPMD).

Strategy (tensor-parallel over output neurons):
  - Each of the 8 blocks' 1024 output neurons is row-sharded 128/core.
  - Masks are folded into weights on the host (memory-bound problem:
    halves HBM traffic vs streaming W and mask separately).
  - All matmuls run in transposed orientation: psum[m, b] += sum_k
    lhsT[k, m] * rhs[k, b] with the (streamed) weights stationary and the
    64-wide batch as the moving operand.  Activations live as x.T
    ([width-on-partition, batch-free]) throughout.
  - Recurrent terms (h @ W_rec[i].T) are independent of the sequential
    chain; they accumulate into per-block PSUM banks at DMA rate while
    the chain (hid + skip + sigmoid + AllGather of the 1024-wide
    activation) threads through.
  - Between blocks: AllGather of the [128, 64] activation shard over the
    8 cores (256 KB total, ~5 us floor).
"""

import numpy as np

N = 8192
W = 1024
L = 8
B = 64
IN = 512
OUT = 512
NCORES = 8
RP = W // NCORES        # 128 rows per core per block
OP = OUT // NCORES      # 64 output rows per core

_BUILT = None


def _pack(A):
    """[M, K] -> [128, (K/128)*M] with packed[p, k*M+m] = A[m, k*128+p].

    Chunk kidx (columns kidx*M:(kidx+1)*M) is A[:, kidx*128:(kidx+1)*128].T,
    i.e. the [K=128(part), M(free)] layout the PE wants for lhsT (stationary)
    or, with M=batch, for rhs (moving).
    """
    M, K = A.shape
    nk = K // 128
    return np.ascontiguousarray(
        A.reshape(M, nk, 128).transpose(2, 1, 0).reshape(128, nk * M)
    )


def _build():
    import concourse.bass as bass
    import concourse.bacc as bacc
    import concourse.mybir as mybir
    import concourse.tile as tile

    fp32 = mybir.dt.float32
    AF = mybir.ActivationFunctionType

    nc = bacc.Bacc(
        "TRN2",
        target_bir_lowering=False,
        debug=False,
        enable_asserts=False,
        num_devices=NCORES,
    )

    t_hT = nc.dram_tensor("hT", [128, 64 * B], fp32, kind="ExternalInput")
    t_xT = nc.dram_tensor("xT", [128, 4 * B], fp32, kind="ExternalInput")
    t_win = nc.dram_tensor("win", [128, 4 * RP], fp32, kind="ExternalInput")
    t_rec = nc.dram_tensor("rec", [7, 128, 64 * RP], fp32, kind="ExternalInput")
    t_hid = nc.dram_tensor("hid", [7, 128, 8 * RP], fp32, kind="ExternalInput")
    t_skip = {
        b: nc.dram_tensor(f"skip{b}", [128, b * 8 * RP], fp32, kind="ExternalInput")
        for b in range(2, 8)
    }
    t_bias = nc.dram_tensor("bias", [128, 8], fp32, kind="ExternalInput")
    t_wout = nc.dram_tensor("wout", [128, 8 * OP], fp32, kind="ExternalInput")
    t_bout = nc.dram_tensor("bout", [OP, 1], fp32, kind="ExternalInput")
    t_out = nc.dram_tensor("out", [OP, B], fp32, kind="ExternalOutput")

    rg = [list(range(NCORES))]

    with tile.TileContext(nc) as tc:
        with (
            tc.tile_pool(name="const", bufs=1) as constp,
            tc.tile_pool(name="wrec", bufs=2) as wrecp,
            tc.tile_pool(name="whid", bufs=2) as whidp,
            tc.tile_pool(name="wskip", bufs=2) as wskipp,
            tc.tile_pool(name="curs", bufs=1) as curp,
            tc.tile_pool(name="psum", bufs=1, space="PSUM") as psump,
            tc.tile_pool(name="dram", bufs=3, space="DRAM") as dramp,
        ):
            # ---- persistent inputs -------------------------------------
            hT_sb = constp.tile([128, 64 * B], fp32, name="hT_sb", tag="hT")
            nc.sync.dma_start(out=hT_sb, in_=t_hT[:, :])
            xT_sb = constp.tile([128, 4 * B], fp32, name="xT_sb", tag="xT")
            nc.sync.dma_start(out=xT_sb, in_=t_xT[:, :])
            win_sb = constp.tile([128, 4 * RP], fp32, name="win_sb", tag="win")
            nc.sync.dma_start(out=win_sb, in_=t_win[:, :])
            bias_sb = constp.tile([128, 8], fp32, name="bias_sb", tag="bias")
            nc.sync.dma_start(out=bias_sb, in_=t_bias[:, :])
            wout_sb = constp.tile([128, 8 * OP], fp32, name="wout_sb", tag="wout")
            nc.sync.dma_start(out=wout_sb, in_=t_wout[:, :])
            bout_sb = constp.tile([OP, 1], fp32, name="bout_sb", tag="bout")
            nc.sync.dma_start(out=bout_sb, in_=t_bout[:, :])

            ps = [
                psump.tile([128, B], fp32, name=f"ps{b}", tag=f"ps{b}")
                for b in range(8)
            ]
            curT = [None] * 8  # gathered activations, [128, 8*B] k-packed

            def allgather(b, src_sb):
                """src_sb [128, B] shard -> curT[b] [128, 8*B] full (k-packed)."""
                agin = dramp.tile([128, B], fp32, name=f"agin{b}", tag="agin")
                agout = dramp.tile([W, B], fp32, name=f"agout{b}", tag="agout")
                nc.gpsimd.dma_start(out=agin, in_=src_sb)
                nc.gpsimd.collective_compute(
                    "AllGather",
                    mybir.AluOpType.bypass,
                    replica_groups=rg,
                    ins=[agin.opt()],
                    outs=[agout.opt()],
                )
                dst = curp.tile([128, 8 * B], fp32, name=f"curT{b}", tag=f"curT{b}")
                nc.sync.dma_start(
                    out=dst.rearrange("p (k b) -> p k b", b=B),
                    in_=agout.rearrange("(k p) b -> p k b", p=128),
                )
                curT[b] = dst

            # ---- block 0: sigmoid(x @ W_in.T + b_in + h @ Wr0.T) -------
            rec_sb = wrecp.tile([128, 64 * RP], fp32, name="rec0_sb", tag="rec")
            nc.sync.dma_start(out=rec_sb, in_=t_rec[0, :, :])
            for kk in range(4):
                nc.tensor.matmul(
                    ps[0],
                    lhsT=win_sb[:, kk * RP:(kk + 1) * RP],
                    rhs=xT_sb[:, kk * B:(kk + 1) * B],
                    start=(kk == 0),
                    stop=False,
                )
            for kk in range(64):
                nc.tensor.matmul(
                    ps[0],
                    lhsT=rec_sb[:, kk * RP:(kk + 1) * RP],
                    rhs=hT_sb[:, kk * B:(kk + 1) * B],
                    start=False,
                    stop=(kk == 63),
                )
            cp = curp.tile([128, B], fp32, name="curpart0", tag="curpart")
            nc.scalar.activation(cp, ps[0], AF.Sigmoid, bias=bias_sb[:, 0:1])
            allgather(0, cp)

            # ---- blocks 1..7 -------------------------------------------
            for b in range(1, 8):
                has_rec = b <= 6
                if has_rec:
                    rec_sb = wrecp.tile(
                        [128, 64 * RP], fp32, name=f"rec{b}_sb", tag="rec"
                    )
                    nc.sync.dma_start(out=rec_sb, in_=t_rec[b, :, :])
                hid_sb = whidp.tile([128, 8 * RP], fp32, name=f"hid{b}_sb", tag="hid")
                nc.sync.dma_start(out=hid_sb, in_=t_hid[b - 1, :, :])
                if b >= 2:
                    skip_sb = wskipp.tile(
                        [128, b * 8 * RP], fp32, name=f"skip{b}_sb", tag="skip"
                    )
                    nc.sync.dma_start(out=skip_sb, in_=t_skip[b][:, :])

                if has_rec:
                    for kk in range(64):
                        nc.tensor.matmul(
                            ps[b],
                            lhsT=rec_sb[:, kk * RP:(kk + 1) * RP],
                            rhs=hT_sb[:, kk * B:(kk + 1) * B],
                            start=(kk == 0),
                            stop=False,
                        )
                if b >= 2:
                    for c in range(b):
                        for kk in range(8):
                            kidx = c * 8 + kk
                            nc.tensor.matmul(
                                ps[b],
                                lhsT=skip_sb[:, kidx * RP:(kidx + 1) * RP],
                                rhs=curT[c][:, kk * B:(kk + 1) * B],
                                start=(not has_rec and kidx == 0),
                                stop=False,
                            )
                for kk in range(8):
                    nc.tensor.matmul(
                        ps[b],
                        lhsT=hid_sb[:, kk * RP:(kk + 1) * RP],
                        rhs=curT[b - 1][:, kk * B:(kk + 1) * B],
                        start=False,
                        stop=(kk == 7),
                    )
                cp = curp.tile([128, B], fp32, name=f"curpart{b}", tag="curpart")
                nc.scalar.activation(cp, ps[b], AF.Sigmoid, bias=bias_sb[:, b:b + 1])
                allgather(b, cp)

            # ---- output block: cur7 @ W_out.T + b_out ------------------
            pso = psump.tile([OP, B], fp32, name="pso", tag="ps0")
            for kk in range(8):
                nc.tensor.matmul(
                    pso,
                    lhsT=wout_sb[:, kk * OP:(kk + 1) * OP],
                    rhs=curT[7][:, kk * B:(kk + 1) * B],
                    start=(kk == 0),
                    stop=(kk == 7),
                )
            out_sb = curp.tile([OP, B], fp32, name="out_sb", tag="out_sb")
            nc.scalar.activation(out_sb, pso, AF.Identity, bias=bout_sb[:, 0:1])
            nc.sync.dma_start(out=t_out[:, :], in_=out_sb)

    nc.compile()
    return nc


def _get_nc():
    global _BUILT
    if _BUILT is None:
        _BUILT = _build()
    return _BUILT


def make_in_maps(x, hidden_states, W_in, b_in, W_hid, b_hid, W_rec, W_skip,
                 W_out, b_out, mask_hid, mask_rec, mask_skip):
    x = np.asarray(x, np.float32)
    h = np.asarray(hidden_states, np.float32)
    W_in = np.asarray(W_in, np.float32)
    b_in = np.asarray(b_in, np.float32)
    W_out = np.asarray(W_out, np.float32)
    b_out = np.asarray(b_out, np.float32)
    Wh = np.asarray(W_hid, np.float32) * np.asarray(mask_hid, np.float32)
    Wr = np.asarray(W_rec, np.float32) * np.asarray(mask_rec, np.float32)
    Ws = np.asarray(W_skip, np.float32) * np.asarray(mask_skip, np.float32)
    b_hid = np.asarray(b_hid, np.float32)

    hT = _pack(h)
    xT = _pack(x)
    in_maps = []
    for c in range(NCORES):
        R = slice(c * RP, (c + 1) * RP)
        Ro = slice(c * OP, (c + 1) * OP)
        m = {
            "hT": hT,
            "xT": xT,
            "win": _pack(W_in[R]),
            "rec": np.stack([_pack(Wr[i, R]) for i in range(7)]),
            "hid": np.stack([_pack(Wh[i, R]) for i in range(7)]),
            "bias": np.ascontiguousarray(
                np.concatenate([b_in[R, None], b_hid[:, R].T], axis=1)
            ),
            "wout": _pack(W_out[Ro]),
            "bout": np.ascontiguousarray(b_out[Ro, None]),
        }
        for b in range(2, 8):
            m[f"skip{b}"] = _pack(Ws[b - 2, R, :b * W])
        in_maps.append(m)
    return in_maps


def run(in_maps, **kw):
    from concourse import bass_utils
    nc = _get_nc()
    return bass_utils.run_bass_kernel_spmd(
        nc, in_maps, core_ids=list(range(NCORES)), **kw
    )


def kernel(**inputs):
    in_maps = make_in_maps(**inputs)
    res = run(in_maps)
    outT = np.concatenate([res.results[c]["out"] for c in range(NCORES)], axis=0)
    return np.ascontiguousarray(outT.T, dtype=np.float32)


# revision 6
# speedup vs baseline: 1.2021x; 1.2021x over previous
"""BrainRNN forward pass on 8 TRN2 NeuronCores (Bass/Tile, SPMD).

Strategy (tensor-parallel over output neurons, fp32 exact):
  - Each block's 1024 output neurons are row-sharded 128/core; masks are
    folded into weights on the host (memory-bound: halves HBM traffic).
  - Matmuls run with the (streamed) weights as the MOVING operand and the
    activations as the 64-wide stationary operand: psum[b, m] += sum_k
    actT[k, b] * W.T[k, m].  fp32 moving runs at ~2 cyc/row, so weight
    ingest is ~350-450 GB/s vs ~150 GB/s for weights-stationary.
  - Adjacent blocks are PAIRED into one [64, 256] PSUM accumulator so the
    recurrent matmuls (independent of the sequential chain) run at N=256.
  - Per-block biases enter via a K=1 "ones x biasrow" matmul that also
    opens (start=True) each accumulator.
  - Chain per block: sigmoid (PSUM->SBUF), PE transpose back to
    [128(m), 64(b)], DMA PSUM->DRAM, AllGather over 8 cores, unpack.
  - Output block: each core contracts its own 128 rows of cur7 against
    W_out[:, rows].T (N=512), then one ReduceScatter yields each core an
    8-row batch shard of the [64, 512] output.
"""

import numpy as np

N = 8192
W = 1024
L = 8
B = 64
IN = 512
OUT = 512
NCORES = 8
RP = W // NCORES        # 128 rows per core per block

_BUILT = None


def _pack(A):
    """[M, K] -> [128, (K/128)*M] with packed[p, k*M+m] = A[m, k*128+p].

    Chunk kidx is A[:, kidx*128:(kidx+1)*128].T, i.e. [K=128(part), M(free)].
    """
    M, K = A.shape
    nk = K // 128
    return np.ascontiguousarray(
        A.reshape(M, nk, 128).transpose(2, 1, 0).reshape(128, nk * M)
    )


def _interleave(Pa, Pb):
    """Two packed [128, nk*128] -> [128, nk*256] with per-chunk interleave."""
    nk = Pa.shape[1] // 128
    out = np.empty((128, nk, 2, 128), np.float32)
    out[:, :, 0, :] = Pa.reshape(128, nk, 128)
    out[:, :, 1, :] = Pb.reshape(128, nk, 128)
    return np.ascontiguousarray(out.reshape(128, nk * 256))


def _build():
    import concourse.bass as bass
    import concourse.bacc as bacc
    import concourse.mybir as mybir
    import concourse.tile as tile

    fp32 = mybir.dt.float32
    AF = mybir.ActivationFunctionType

    nc = bacc.Bacc(
        "TRN2",
        target_bir_lowering=False,
        debug=False,
        enable_asserts=False,
        num_devices=NCORES,
    )

    t_hT = nc.dram_tensor("hT", [128, 64 * B], fp32, kind="ExternalInput")
    t_xT = nc.dram_tensor("xT", [128, 4 * B], fp32, kind="ExternalInput")
    t_win = nc.dram_tensor("win", [128, 4 * RP], fp32, kind="ExternalInput")
    t_rec = [
        nc.dram_tensor("rec0", [128, 64 * 256], fp32, kind="ExternalInput"),
        nc.dram_tensor("rec1", [128, 64 * 256], fp32, kind="ExternalInput"),
        nc.dram_tensor("rec2", [128, 64 * 256], fp32, kind="ExternalInput"),
        nc.dram_tensor("rec3", [128, 64 * 128], fp32, kind="ExternalInput"),
    ]
    t_hid = nc.dram_tensor("hid", [7, 128, 8 * RP], fp32, kind="ExternalInput")
    # skip{q}_{c}: pair q=(2q, 2q+1), cur-source block c.  c < 2q: both
    # blocks interleaved [128, 8*256]; c == 2q: later block only [128, 8*128].
    t_skip = {}
    for q in range(1, 4):
        a = 2 * q
        for c in range(a + 1):
            wdt = 8 * 256 if c < a else 8 * 128
            t_skip[(q, c)] = nc.dram_tensor(
                f"skip{q}_{c}", [128, wdt], fp32, kind="ExternalInput"
            )
    t_biasrow = nc.dram_tensor("biasrow", [1, 4 * 256], fp32, kind="ExternalInput")
    t_wout = nc.dram_tensor("wout", [128, 512], fp32, kind="ExternalInput")
    t_boutrow = nc.dram_tensor("boutrow", [1, 512], fp32, kind="ExternalInput")
    t_ones = nc.dram_tensor("ones", [1, B], fp32, kind="ExternalInput")
    t_ident = nc.dram_tensor("ident", [B, B], fp32, kind="ExternalInput")
    t_out = nc.dram_tensor("out", [8, 512], fp32, kind="ExternalOutput")

    rg = [list(range(NCORES))]
    qof = lambda j: j // 2          # pair index of block j
    side = lambda j: j % 2          # column side within pair tile

    with tile.TileContext(nc) as tc:
        with (
            tc.tile_pool(name="const", bufs=1) as constp,
            tc.tile_pool(name="wrec", bufs=3) as wrecp,
            tc.tile_pool(name="whid", bufs=2) as whidp,
            tc.tile_pool(name="wskip", bufs=4) as wskipp,
            tc.tile_pool(name="curs", bufs=1) as curp,
            tc.tile_pool(name="psum", bufs=1, space="PSUM") as psump,
            tc.tile_pool(name="dram", bufs=3, space="DRAM") as dramp,
        ):
            # ---- persistent inputs -------------------------------------
            hT_sb = constp.tile([128, 64 * B], fp32, name="hT_sb", tag="hT")
            nc.sync.dma_start(out=hT_sb, in_=t_hT[:, :])
            xT_sb = constp.tile([128, 4 * B], fp32, name="xT_sb", tag="xT")
            nc.sync.dma_start(out=xT_sb, in_=t_xT[:, :])
            win_sb = constp.tile([128, 4 * RP], fp32, name="win_sb", tag="win")
            nc.sync.dma_start(out=win_sb, in_=t_win[:, :])
            biasrow_sb = constp.tile([1, 4 * 256], fp32, name="biasrow_sb", tag="br")
            nc.sync.dma_start(out=biasrow_sb, in_=t_biasrow[:, :])
            wout_sb = constp.tile([128, 512], fp32, name="wout_sb", tag="wout")
            nc.sync.dma_start(out=wout_sb, in_=t_wout[:, :])
            boutrow_sb = constp.tile([1, 512], fp32, name="boutrow_sb", tag="bo")
            nc.sync.dma_start(out=boutrow_sb, in_=t_boutrow[:, :])
            ones_sb = constp.tile([1, B], fp32, name="ones_sb", tag="ones")
            nc.sync.dma_start(out=ones_sb, in_=t_ones[:, :])
            ident_sb = constp.tile([B, B], fp32, name="ident_sb", tag="ident")
            nc.sync.dma_start(out=ident_sb, in_=t_ident[:, :])

            psA = [
                psump.tile([64, 256], fp32, name=f"psA{q}", tag=f"psA{q}")
                for q in range(4)
            ]
            curT = [None] * 8

            # bias-init: psA[q] = ones.T @ biasrow[q]  (start=True opens group)
            for q in range(4):
                nc.tensor.matmul(
                    psA[q],
                    lhsT=ones_sb[:, :],
                    rhs=biasrow_sb[:, q * 256:(q + 1) * 256],
                    start=True,
                    stop=False,
                )

            def rec_pair(q):
                """Stream pair q's rec weights (2 x 4MB halves) + matmuls."""
                wide = 256 if q < 3 else 128
                nhalf = 2 if q < 3 else 1
                for h in range(nhalf):
                    rt = wrecp.tile([128, 8192], fp32, name=f"rec{q}h{h}", tag="rec")
                    nc.sync.dma_start(
                        out=rt, in_=t_rec[q][:, h * 8192:(h + 1) * 8192]
                    )
                    kn = 8192 // wide
                    for k in range(kn):
                        kg = h * kn + k
                        nc.tensor.matmul(
                            psA[q][:, 0:wide],
                            lhsT=hT_sb[:, kg * B:(kg + 1) * B],
                            rhs=rt[:, k * wide:(k + 1) * wide],
                            start=False,
                            stop=False,
                        )

            def chain_tail(j, last_of_tile):
                """sigmoid -> transpose -> AllGather -> unpack for block j."""
                q, s = qof(j), side(j)
                cp = curp.tile([64, 128], fp32, name=f"cpart{j}", tag="cpart", bufs=2)
                nc.scalar.activation(cp, psA[q][:, s * 128:(s + 1) * 128], AF.Sigmoid)
                pt = psump.tile([128, B], fp32, name=f"pt{j}", tag="pt", bufs=2)
                nc.tensor.transpose(pt, cp, ident_sb[:, :])
                ptsb = curp.tile([128, B], fp32, name=f"ptsb{j}", tag="ptsb", bufs=2)
                nc.vector.tensor_copy(ptsb, pt)
                agin = dramp.tile([128, B], fp32, name=f"agin{j}", tag="agin")
                agout = dramp.tile([W, B], fp32, name=f"agout{j}", tag="agout")
                nc.sync.dma_start(out=agin, in_=ptsb)
                nc.gpsimd.collective_compute(
                    "AllGather",
                    mybir.AluOpType.bypass,
                    replica_groups=rg,
                    ins=[agin.opt()],
                    outs=[agout.opt()],
                )
                dst = curp.tile([128, 8 * B], fp32, name=f"curT{j}", tag=f"curT{j}")
                nc.sync.dma_start(
                    out=dst.rearrange("p (k b) -> p k b", b=B),
                    in_=agout.rearrange("(k p) b -> p k b", p=128),
                )
                curT[j] = dst

            def hid_mms(j):
                """cur_{j-1} @ W_hid[j-1].T into block j's psum columns."""
                q, s = qof(j), side(j)
                ht = whidp.tile([128, 8 * RP], fp32, name=f"hid{j}", tag="hid")
                nc.sync.dma_start(out=ht, in_=t_hid[j - 1, :, :])
                for kk in range(8):
                    nc.tensor.matmul(
                        psA[q][:, s * 128:(s + 1) * 128],
                        lhsT=curT[j - 1][:, kk * B:(kk + 1) * B],
                        rhs=ht[:, kk * RP:(kk + 1) * RP],
                        start=False,
                        stop=(s == 1 and kk == 7),
                    )

            def skip_mms(c):
                """All pairs' skip contributions sourced from cur_c."""
                for q in range(1, 4):
                    a = 2 * q
                    if c > a:
                        continue
                    wide = 256 if c < a else 128
                    st = wskipp.tile(
                        [128, 8 * wide], fp32, name=f"skip{q}_{c}t", tag="skip"
                    )
                    nc.sync.dma_start(out=st, in_=t_skip[(q, c)][:, :])
                    off = 0 if c < a else 128
                    for kk in range(8):
                        nc.tensor.matmul(
                            psA[q][:, off:off + wide],
                            lhsT=curT[c][:, kk * B:(kk + 1) * B],
                            rhs=st[:, kk * wide:(kk + 1) * wide],
                            start=False,
                            stop=False,
                        )

            # ---- block 0: x @ W_in.T + h @ Wr0.T (+bias) ---------------
            for kk in range(4):
                nc.tensor.matmul(
                    psA[0][:, 0:128],
                    lhsT=xT_sb[:, kk * B:(kk + 1) * B],
                    rhs=win_sb[:, kk * RP:(kk + 1) * RP],
                    start=False,
                    stop=False,
                )
            rec_pair(0)
            chain_tail(0, False)

            rec_pair(1)
            hid_mms(1)
            chain_tail(1, True)

            rec_pair(2)
            skip_mms(0)
            skip_mms(1)
            hid_mms(2)
            chain_tail(2, False)

            skip_mms(2)
            hid_mms(3)
            chain_tail(3, True)

            rec_pair(3)
            skip_mms(3)
            hid_mms(4)
            chain_tail(4, False)

            skip_mms(4)
            hid_mms(5)
            chain_tail(5, True)

            skip_mms(5)
            hid_mms(6)
            chain_tail(6, False)

            skip_mms(6)
            hid_mms(7)
            # block 7 tail: sigmoid -> transpose -> local out matmul -> RS
            cp7 = curp.tile([64, 128], fp32, name="cpart7", tag="cpart", bufs=2)
            nc.scalar.activation(cp7, psA[3][:, 128:256], AF.Sigmoid)
            pt7 = psump.tile([128, B], fp32, name="pt7", tag="pt", bufs=2)
            nc.tensor.transpose(pt7, cp7, ident_sb[:, :])
            cur7T_sb = curp.tile([128, B], fp32, name="cur7T_sb", tag="c7T")
            nc.vector.tensor_copy(cur7T_sb, pt7)

            pso = psump.tile([64, 512], fp32, name="pso", tag="pso")
            nc.tensor.matmul(
                pso, lhsT=ones_sb[:, :], rhs=boutrow_sb[:, :], start=True, stop=False
            )
            nc.tensor.matmul(
                pso, lhsT=cur7T_sb, rhs=wout_sb[:, :], start=False, stop=True
            )
            out_sb = curp.tile([64, 512], fp32, name="out_sb", tag="out_sb")
            nc.vector.tensor_copy(out_sb, pso)
            rs_in = dramp.tile([64, 512], fp32, name="rs_in", tag="rs_in")
            rs_out = dramp.tile([8, 512], fp32, name="rs_out", tag="rs_out")
            nc.sync.dma_start(out=rs_in, in_=out_sb)
            nc.gpsimd.collective_compute(
                "ReduceScatter",
                mybir.AluOpType.add,
                replica_groups=rg,
                ins=[rs_in.opt()],
                outs=[rs_out.opt()],
            )
            nc.sync.dma_start(out=t_out[:, :], in_=rs_out)

    nc.compile()
    return nc


def _get_nc():
    global _BUILT
    if _BUILT is None:
        _BUILT = _build()
    return _BUILT


def make_in_maps(x, hidden_states, W_in, b_in, W_hid, b_hid, W_rec, W_skip,
                 W_out, b_out, mask_hid, mask_rec, mask_skip):
    x = np.asarray(x, np.float32)
    h = np.asarray(hidden_states, np.float32)
    W_in = np.asarray(W_in, np.float32)
    b_in = np.asarray(b_in, np.float32)
    W_out = np.asarray(W_out, np.float32)
    b_out = np.asarray(b_out, np.float32)
    Wh = np.asarray(W_hid, np.float32) * np.asarray(mask_hid, np.float32)
    Wr = np.asarray(W_rec, np.float32) * np.asarray(mask_rec, np.float32)
    Ws = np.asarray(W_skip, np.float32) * np.asarray(mask_skip, np.float32)
    b_hid = np.asarray(b_hid, np.float32)

    hT = _pack(h)
    xT = _pack(x)
    ones = np.ones((1, B), np.float32)
    ident = np.eye(B, dtype=np.float32)
    # every core's partial includes the bias and ReduceScatter sums them
    boutrow = np.ascontiguousarray(b_out[None, :]) / NCORES

    in_maps = []
    for c_ in range(NCORES):
        R = slice(c_ * RP, (c_ + 1) * RP)
        recs = [_pack(Wr[i, R]) for i in range(7)]
        biases = [b_in[R]] + [b_hid[i, R] for i in range(7)]
        biasrow = np.zeros((1, 4 * 256), np.float32)
        for j in range(8):
            biasrow[0, j * 128:(j + 1) * 128] = biases[j]
        m = {
            "hT": hT,
            "xT": xT,
            "win": _pack(W_in[R]),
            "rec0": _interleave(recs[0], recs[1]),
            "rec1": _interleave(recs[2], recs[3]),
            "rec2": _interleave(recs[4], recs[5]),
            "rec3": recs[6],
            "hid": np.stack([_pack(Wh[i, R]) for i in range(7)]),
            "biasrow": biasrow,
            "wout": np.ascontiguousarray(W_out[:, R].T),
            "boutrow": boutrow,
            "ones": ones,
            "ident": ident,
        }
        packs = {j: _pack(Ws[j - 2, R, :j * W]).reshape(128, j * 8, 128)
                 for j in range(2, 8)}
        for q in range(1, 4):
            a = 2 * q
            for c in range(a + 1):
                if c < a:
                    Pa = packs[a][:, c * 8:(c + 1) * 8, :].reshape(128, 8 * 128)
                    Pb = packs[a + 1][:, c * 8:(c + 1) * 8, :].reshape(128, 8 * 128)
                    m[f"skip{q}_{c}"] = _interleave(
                        np.ascontiguousarray(Pa), np.ascontiguousarray(Pb)
                    )
                else:
                    m[f"skip{q}_{c}"] = np.ascontiguousarray(
                        packs[a + 1][:, c * 8:(c + 1) * 8, :].reshape(128, 8 * 128)
                    )
        in_maps.append(m)
    return in_maps


def run(in_maps, **kw):
    from concourse import bass_utils
    nc = _get_nc()
    return bass_utils.run_bass_kernel_spmd(
        nc, in_maps, core_ids=list(range(NCORES)), **kw
    )


def kernel(**inputs):
    in_maps = make_in_maps(**inputs)
    res = run(in_maps)
    return np.ascontiguousarray(
        np.concatenate([res.results[c]["out"] for c in range(NCORES)], axis=0),
        dtype=np.float32,
    )


# revision 12
# speedup vs baseline: 1.2165x; 1.0120x over previous
"""BrainRNN forward pass on 8 TRN2 NeuronCores (Bass/Tile, SPMD).

Strategy (tensor-parallel over output neurons, fp32 exact):
  - Each block's 1024 output neurons are row-sharded 128/core; masks are
    folded into weights on the host (memory-bound: halves HBM traffic).
  - Matmuls run with the (streamed) weights as the MOVING operand and the
    activations as the 64-wide stationary operand: psum[b, m] += sum_k
    actT[k, b] * W.T[k, m].  fp32 moving runs at ~2 cyc/row, so weight
    ingest is ~350-450 GB/s vs ~150 GB/s for weights-stationary.
  - Adjacent blocks are PAIRED into one [64, 256] PSUM accumulator so the
    recurrent matmuls (independent of the sequential chain) run at N=256.
  - Per-block biases enter via a K=1 "ones x biasrow" matmul that also
    opens (start=True) each accumulator.
  - Chain per block: sigmoid (PSUM->SBUF), PE transpose back to
    [128(m), 64(b)], DMA PSUM->DRAM, AllGather over 8 cores, unpack.
  - Output block: each core contracts its own 128 rows of cur7 against
    W_out[:, rows].T (N=512), then one ReduceScatter yields each core an
    8-row batch shard of the [64, 512] output.
"""

import numpy as np

N = 8192
W = 1024
L = 8
B = 64
IN = 512
OUT = 512
NCORES = 8
RP = W // NCORES        # 128 rows per core per block

_BUILT = None


def _pack(A):
    """[M, K] -> [128, (K/128)*M] with packed[p, k*M+m] = A[m, k*128+p].

    Chunk kidx is A[:, kidx*128:(kidx+1)*128].T, i.e. [K=128(part), M(free)].
    """
    M, K = A.shape
    nk = K // 128
    return np.ascontiguousarray(
        A.reshape(M, nk, 128).transpose(2, 1, 0).reshape(128, nk * M)
    )


def _interleave(Pa, Pb):
    """Two packed [128, nk*128] -> [128, nk*256] with per-chunk interleave."""
    nk = Pa.shape[1] // 128
    out = np.empty((128, nk, 2, 128), np.float32)
    out[:, :, 0, :] = Pa.reshape(128, nk, 128)
    out[:, :, 1, :] = Pb.reshape(128, nk, 128)
    return np.ascontiguousarray(out.reshape(128, nk * 256))


def _build():
    import concourse.bass as bass
    import concourse.bacc as bacc
    import concourse.mybir as mybir
    import concourse.tile as tile

    fp32 = mybir.dt.float32
    AF = mybir.ActivationFunctionType

    nc = bacc.Bacc(
        "TRN2",
        target_bir_lowering=False,
        debug=False,
        enable_asserts=False,
        num_devices=NCORES,
    )

    t_hT = nc.dram_tensor("hT", [128, 64 * B], fp32, kind="ExternalInput")
    t_xT = nc.dram_tensor("xT", [128, 4 * B], fp32, kind="ExternalInput")
    t_win = nc.dram_tensor("win", [128, 4 * RP], fp32, kind="ExternalInput")
    t_rec = [
        nc.dram_tensor("rec0", [128, 64 * 256], fp32, kind="ExternalInput"),
        nc.dram_tensor("rec1", [128, 64 * 256], fp32, kind="ExternalInput"),
        nc.dram_tensor("rec2", [128, 64 * 256], fp32, kind="ExternalInput"),
        nc.dram_tensor("rec3", [128, 64 * 128], fp32, kind="ExternalInput"),
    ]
    t_hid = nc.dram_tensor("hid", [7, 128, 8 * RP], fp32, kind="ExternalInput")
    # skip{q}_{c}: pair q=(2q, 2q+1), cur-source block c.  c < 2q: both
    # blocks interleaved [128, 8*256]; c == 2q: later block only [128, 8*128].
    t_skip = {}
    for q in range(1, 4):
        a = 2 * q
        for c in range(a + 1):
            wdt = 8 * 256 if c < a else 8 * 128
            t_skip[(q, c)] = nc.dram_tensor(
                f"skip{q}_{c}", [128, wdt], fp32, kind="ExternalInput"
            )
    t_biasrow = nc.dram_tensor("biasrow", [1, 4 * 256], fp32, kind="ExternalInput")
    t_wout = nc.dram_tensor("wout", [128, 512], fp32, kind="ExternalInput")
    t_boutrow = nc.dram_tensor("boutrow", [1, 512], fp32, kind="ExternalInput")
    t_ones = nc.dram_tensor("ones", [1, B], fp32, kind="ExternalInput")
    t_ident = nc.dram_tensor("ident", [B, B], fp32, kind="ExternalInput")
    t_out = nc.dram_tensor("out", [8, 512], fp32, kind="ExternalOutput")

    rg = [list(range(NCORES))]
    qof = lambda j: j // 2          # pair index of block j
    side = lambda j: j % 2          # column side within pair tile

    with tile.TileContext(nc) as tc:
        with (
            tc.tile_pool(name="const", bufs=1) as constp,
            tc.tile_pool(name="wrec", bufs=3) as wrecp,
            tc.tile_pool(name="whid", bufs=2) as whidp,
            tc.tile_pool(name="wskip", bufs=4) as wskipp,
            tc.tile_pool(name="curs", bufs=1) as curp,
            tc.tile_pool(name="psum", bufs=1, space="PSUM") as psump,
            tc.tile_pool(name="dram", bufs=3, space="DRAM") as dramp,
        ):
            # ---- persistent inputs -------------------------------------
            hT_sb = constp.tile([128, 64 * B], fp32, name="hT_sb", tag="hT")
            nc.sync.dma_start(out=hT_sb, in_=t_hT[:, :])
            xT_sb = constp.tile([128, 4 * B], fp32, name="xT_sb", tag="xT")
            nc.sync.dma_start(out=xT_sb, in_=t_xT[:, :])
            win_sb = constp.tile([128, 4 * RP], fp32, name="win_sb", tag="win")
            nc.sync.dma_start(out=win_sb, in_=t_win[:, :])
            biasrow_sb = constp.tile([1, 4 * 256], fp32, name="biasrow_sb", tag="br")
            nc.sync.dma_start(out=biasrow_sb, in_=t_biasrow[:, :])
            wout_sb = constp.tile([128, 512], fp32, name="wout_sb", tag="wout")
            nc.sync.dma_start(out=wout_sb, in_=t_wout[:, :])
            boutrow_sb = constp.tile([1, 512], fp32, name="boutrow_sb", tag="bo")
            nc.sync.dma_start(out=boutrow_sb, in_=t_boutrow[:, :])
            ones_sb = constp.tile([1, B], fp32, name="ones_sb", tag="ones")
            nc.sync.dma_start(out=ones_sb, in_=t_ones[:, :])
            ident_sb = constp.tile([B, B], fp32, name="ident_sb", tag="ident")
            nc.sync.dma_start(out=ident_sb, in_=t_ident[:, :])

            psA = [
                psump.tile([64, 256], fp32, name=f"psA{q}", tag=f"psA{q}")
                for q in range(4)
            ]
            curT = [None] * 8

            # bias-init: psA[q] = ones.T @ biasrow[q]  (start=True opens group)
            for q in range(4):
                nc.tensor.matmul(
                    psA[q],
                    lhsT=ones_sb[:, :],
                    rhs=biasrow_sb[:, q * 256:(q + 1) * 256],
                    start=True,
                    stop=False,
                )

            def rec_pair(q):
                """Stream pair q's rec weights (1MB DMA chunks) + matmuls."""
                wide = 256 if q < 3 else 128
                nhalf = 2 if q < 3 else 1
                for h in range(nhalf):
                    rt = wrecp.tile([128, 8192], fp32, name=f"rec{q}h{h}", tag="rec")
                    for i in range(4):
                        nc.sync.dma_start(
                            out=rt[:, i * 2048:(i + 1) * 2048],
                            in_=t_rec[q][:, h * 8192 + i * 2048:
                                         h * 8192 + (i + 1) * 2048],
                        )
                    kn = 8192 // wide
                    for k in range(kn):
                        kg = h * kn + k
                        nc.tensor.matmul(
                            psA[q][:, 0:wide],
                            lhsT=hT_sb[:, kg * B:(kg + 1) * B],
                            rhs=rt[:, k * wide:(k + 1) * wide],
                            start=False,
                            stop=False,
                        )

            def chain_tail(j, last_of_tile):
                """sigmoid -> transpose -> AllGather -> unpack for block j."""
                q, s = qof(j), side(j)
                cp = curp.tile([64, 128], fp32, name=f"cpart{j}", tag="cpart", bufs=2)
                nc.scalar.activation(cp, psA[q][:, s * 128:(s + 1) * 128], AF.Sigmoid)
                pt = psump.tile([128, B], fp32, name=f"pt{j}", tag="pt", bufs=2)
                nc.tensor.transpose(pt, cp, ident_sb[:, :])
                ptsb = curp.tile([128, B], fp32, name=f"ptsb{j}", tag="ptsb", bufs=2)
                nc.vector.tensor_copy(ptsb, pt)
                agin = dramp.tile([128, B], fp32, name=f"agin{j}", tag="agin")
                agout = dramp.tile([W, B], fp32, name=f"agout{j}", tag="agout")
                nc.scalar.dma_start(out=agin, in_=ptsb)
                nc.gpsimd.collective_compute(
                    "AllGather",
                    mybir.AluOpType.bypass,
                    replica_groups=rg,
                    ins=[agin.opt()],
                    outs=[agout.opt()],
                )
                dst = curp.tile([128, 8 * B], fp32, name=f"curT{j}", tag=f"curT{j}")
                for kk in range(8):
                    nc.scalar.dma_start(
                        out=dst[:, kk * B:(kk + 1) * B],
                        in_=agout[kk * 128:(kk + 1) * 128, :],
                    )
                curT[j] = dst

            def hid_mms(j):
                """cur_{j-1} @ W_hid[j-1].T into block j's psum columns."""
                q, s = qof(j), side(j)
                ht = whidp.tile([128, 8 * RP], fp32, name=f"hid{j}", tag="hid")
                nc.sync.dma_start(out=ht, in_=t_hid[j - 1, :, :])
                for kk in range(8):
                    nc.tensor.matmul(
                        psA[q][:, s * 128:(s + 1) * 128],
                        lhsT=curT[j - 1][:, kk * B:(kk + 1) * B],
                        rhs=ht[:, kk * RP:(kk + 1) * RP],
                        start=False,
                        stop=(s == 1 and kk == 7),
                    )

            def skip_one(q, c):
                """Pair q's skip contribution sourced from cur_c."""
                a = 2 * q
                wide = 256 if c < a else 128
                st = wskipp.tile(
                    [128, 8 * wide], fp32, name=f"skip{q}_{c}t", tag="skip"
                )
                nc.sync.dma_start(out=st, in_=t_skip[(q, c)][:, :])
                off = 0 if c < a else 128
                for kk in range(8):
                    nc.tensor.matmul(
                        psA[q][:, off:off + wide],
                        lhsT=curT[c][:, kk * B:(kk + 1) * B],
                        rhs=st[:, kk * wide:(kk + 1) * wide],
                        start=False,
                        stop=False,
                    )

            # ---- block 0: x @ W_in.T + h @ Wr0.T (+bias) ---------------
            for kk in range(4):
                nc.tensor.matmul(
                    psA[0][:, 0:128],
                    lhsT=xT_sb[:, kk * B:(kk + 1) * B],
                    rhs=win_sb[:, kk * RP:(kk + 1) * RP],
                    start=False,
                    stop=False,
                )
            rec_pair(0)
            chain_tail(0, False)

            rec_pair(1)
            hid_mms(1)
            chain_tail(1, True)
            # lazy skips sourced from cur_0 (feed blocks 2..7)
            skip_one(1, 0)
            skip_one(2, 0)
            skip_one(3, 0)

            rec_pair(2)
            skip_one(1, 1)          # urgent: block 2's last skip source
            hid_mms(2)
            chain_tail(2, False)
            skip_one(2, 1)
            skip_one(3, 1)

            skip_one(1, 2)          # urgent for block 3
            hid_mms(3)
            chain_tail(3, True)
            skip_one(2, 2)
            skip_one(3, 2)

            rec_pair(3)
            skip_one(2, 3)          # urgent for block 4
            hid_mms(4)
            chain_tail(4, False)
            skip_one(3, 3)

            skip_one(2, 4)          # urgent for block 5
            hid_mms(5)
            chain_tail(5, True)
            skip_one(3, 4)

            skip_one(3, 5)          # urgent for block 6 (covers 6 and 7)
            hid_mms(6)
            chain_tail(6, False)

            skip_one(3, 6)          # urgent for block 7
            hid_mms(7)
            # block 7 tail: sigmoid -> transpose -> local out matmul -> RS
            cp7 = curp.tile([64, 128], fp32, name="cpart7", tag="cpart", bufs=2)
            nc.scalar.activation(cp7, psA[3][:, 128:256], AF.Sigmoid)
            pt7 = psump.tile([128, B], fp32, name="pt7", tag="pt", bufs=2)
            nc.tensor.transpose(pt7, cp7, ident_sb[:, :])
            cur7T_sb = curp.tile([128, B], fp32, name="cur7T_sb", tag="c7T")
            nc.vector.tensor_copy(cur7T_sb, pt7)

            pso = psump.tile([64, 512], fp32, name="pso", tag="pso")
            nc.tensor.matmul(
                pso, lhsT=ones_sb[:, :], rhs=boutrow_sb[:, :], start=True, stop=False
            )
            nc.tensor.matmul(
                pso, lhsT=cur7T_sb, rhs=wout_sb[:, :], start=False, stop=True
            )
            out_sb = curp.tile([64, 512], fp32, name="out_sb", tag="out_sb")
            nc.vector.tensor_copy(out_sb, pso)
            rs_in = dramp.tile([64, 512], fp32, name="rs_in", tag="rs_in")
            rs_out = dramp.tile([8, 512], fp32, name="rs_out", tag="rs_out")
            nc.scalar.dma_start(out=rs_in, in_=out_sb)
            nc.gpsimd.collective_compute(
                "ReduceScatter",
                mybir.AluOpType.add,
                replica_groups=rg,
                ins=[rs_in.opt()],
                outs=[rs_out.opt()],
            )
            nc.scalar.dma_start(out=t_out[:, :], in_=rs_out)

    nc.compile()
    return nc


def _get_nc():
    global _BUILT
    if _BUILT is None:
        _BUILT = _build()
    return _BUILT


def make_in_maps(x, hidden_states, W_in, b_in, W_hid, b_hid, W_rec, W_skip,
                 W_out, b_out, mask_hid, mask_rec, mask_skip):
    x = np.asarray(x, np.float32)
    h = np.asarray(hidden_states, np.float32)
    W_in = np.asarray(W_in, np.float32)
    b_in = np.asarray(b_in, np.float32)
    W_out = np.asarray(W_out, np.float32)
    b_out = np.asarray(b_out, np.float32)
    Wh = np.asarray(W_hid, np.float32) * np.asarray(mask_hid, np.float32)
    Wr = np.asarray(W_rec, np.float32) * np.asarray(mask_rec, np.float32)
    Ws = np.asarray(W_skip, np.float32) * np.asarray(mask_skip, np.float32)
    b_hid = np.asarray(b_hid, np.float32)

    hT = _pack(h)
    xT = _pack(x)
    ones = np.ones((1, B), np.float32)
    ident = np.eye(B, dtype=np.float32)
    # every core's partial includes the bias and ReduceScatter sums them
    boutrow = np.ascontiguousarray(b_out[None, :]) / NCORES

    in_maps = []
    for c_ in range(NCORES):
        R = slice(c_ * RP, (c_ + 1) * RP)
        recs = [_pack(Wr[i, R]) for i in range(7)]
        biases = [b_in[R]] + [b_hid[i, R] for i in range(7)]
        biasrow = np.zeros((1, 4 * 256), np.float32)
        for j in range(8):
            biasrow[0, j * 128:(j + 1) * 128] = biases[j]
        m = {
            "hT": hT,
            "xT": xT,
            "win": _pack(W_in[R]),
            "rec0": _interleave(recs[0], recs[1]),
            "rec1": _interleave(recs[2], recs[3]),
            "rec2": _interleave(recs[4], recs[5]),
            "rec3": recs[6],
            "hid": np.stack([_pack(Wh[i, R]) for i in range(7)]),
            "biasrow": biasrow,
            "wout": np.ascontiguousarray(W_out[:, R].T),
            "boutrow": boutrow,
            "ones": ones,
            "ident": ident,
        }
        packs = {j: _pack(Ws[j - 2, R, :j * W]).reshape(128, j * 8, 128)
                 for j in range(2, 8)}
        for q in range(1, 4):
            a = 2 * q
            for c in range(a + 1):
                if c < a:
                    Pa = packs[a][:, c * 8:(c + 1) * 8, :].reshape(128, 8 * 128)
                    Pb = packs[a + 1][:, c * 8:(c + 1) * 8, :].reshape(128, 8 * 128)
                    m[f"skip{q}_{c}"] = _interleave(
                        np.ascontiguousarray(Pa), np.ascontiguousarray(Pb)
                    )
                else:
                    m[f"skip{q}_{c}"] = np.ascontiguousarray(
                        packs[a + 1][:, c * 8:(c + 1) * 8, :].reshape(128, 8 * 128)
                    )
        in_maps.append(m)
    return in_maps


def run(in_maps, **kw):
    from concourse import bass_utils
    nc = _get_nc()
    return bass_utils.run_bass_kernel_spmd(
        nc, in_maps, core_ids=list(range(NCORES)), **kw
    )


def kernel(**inputs):
    in_maps = make_in_maps(**inputs)
    res = run(in_maps)
    return np.ascontiguousarray(
        np.concatenate([res.results[c]["out"] for c in range(NCORES)], axis=0),
        dtype=np.float32,
    )


# revision 15
# speedup vs baseline: 1.2507x; 1.0282x over previous
"""BrainRNN forward pass on 8 TRN2 NeuronCores (Bass/Tile, SPMD).

Strategy (tensor-parallel over output neurons, fp32 exact):
  - Each block's 1024 output neurons are row-sharded 128/core; masks are
    folded into weights on the host (memory-bound: halves HBM traffic).
  - Matmuls run with the (streamed) weights as the MOVING operand and the
    activations as the 64-wide stationary operand: psum[b, m] += sum_k
    actT[k, b] * W.T[k, m].  fp32 moving runs at ~2 cyc/row, so weight
    ingest is ~350-450 GB/s vs ~150 GB/s for weights-stationary.
  - Adjacent blocks are PAIRED into one [64, 256] PSUM accumulator so the
    recurrent matmuls (independent of the sequential chain) run at N=256.
  - Per-block biases enter via a K=1 "ones x biasrow" matmul that also
    opens (start=True) each accumulator.
  - Chain per block: sigmoid (PSUM->SBUF), PE transpose back to
    [128(m), 64(b)], DMA PSUM->DRAM, AllGather over 8 cores, unpack.
  - Output block: each core contracts its own 128 rows of cur7 against
    W_out[:, rows].T (N=512), then one ReduceScatter yields each core an
    8-row batch shard of the [64, 512] output.
"""

import numpy as np

N = 8192
W = 1024
L = 8
B = 64
IN = 512
OUT = 512
NCORES = 8
RP = W // NCORES        # 128 rows per core per block

_BUILT = None


def _pack(A):
    """[M, K] -> [128, (K/128)*M] with packed[p, k*M+m] = A[m, k*128+p].

    Chunk kidx is A[:, kidx*128:(kidx+1)*128].T, i.e. [K=128(part), M(free)].
    """
    M, K = A.shape
    nk = K // 128
    return np.ascontiguousarray(
        A.reshape(M, nk, 128).transpose(2, 1, 0).reshape(128, nk * M)
    )


def _interleave(Pa, Pb):
    """Two packed [128, nk*128] -> [128, nk*256] with per-chunk interleave."""
    nk = Pa.shape[1] // 128
    out = np.empty((128, nk, 2, 128), np.float32)
    out[:, :, 0, :] = Pa.reshape(128, nk, 128)
    out[:, :, 1, :] = Pb.reshape(128, nk, 128)
    return np.ascontiguousarray(out.reshape(128, nk * 256))


def _build():
    import concourse.bass as bass
    import concourse.bacc as bacc
    import concourse.mybir as mybir
    import concourse.tile as tile

    fp32 = mybir.dt.float32
    AF = mybir.ActivationFunctionType

    nc = bacc.Bacc(
        "TRN2",
        target_bir_lowering=False,
        debug=False,
        enable_asserts=False,
        num_devices=NCORES,
    )

    t_hT = nc.dram_tensor("hT", [128, 64 * B], fp32, kind="ExternalInput")
    t_xT = nc.dram_tensor("xT", [128, 4 * B], fp32, kind="ExternalInput")
    t_win = nc.dram_tensor("win", [128, 4 * RP], fp32, kind="ExternalInput")
    t_rec = [
        nc.dram_tensor("rec0", [128, 64 * 256], fp32, kind="ExternalInput"),
        nc.dram_tensor("rec1", [128, 64 * 256], fp32, kind="ExternalInput"),
        nc.dram_tensor("rec2", [128, 64 * 256], fp32, kind="ExternalInput"),
        nc.dram_tensor("rec3", [128, 64 * 128], fp32, kind="ExternalInput"),
    ]
    t_hid = nc.dram_tensor("hid", [7, 128, 8 * RP], fp32, kind="ExternalInput")
    # skip{q}_{c}: pair q=(2q, 2q+1), cur-source block c.  c < 2q: both
    # blocks interleaved [128, 8*256]; c == 2q: later block only [128, 8*128].
    t_skip = {}
    for q in range(1, 4):
        a = 2 * q
        for c in range(a + 1):
            wdt = 8 * 256 if c < a else 8 * 128
            t_skip[(q, c)] = nc.dram_tensor(
                f"skip{q}_{c}", [128, wdt], fp32, kind="ExternalInput"
            )
    t_biasrow = nc.dram_tensor("biasrow", [1, 4 * 256], fp32, kind="ExternalInput")
    t_wout = nc.dram_tensor("wout", [128, 512], fp32, kind="ExternalInput")
    t_boutrow = nc.dram_tensor("boutrow", [1, 512], fp32, kind="ExternalInput")
    t_ones = nc.dram_tensor("ones", [1, B], fp32, kind="ExternalInput")
    t_ident = nc.dram_tensor("ident", [B, B], fp32, kind="ExternalInput")
    t_out = nc.dram_tensor("out", [8, 512], fp32, kind="ExternalOutput")

    rg = [list(range(NCORES))]
    qof = lambda j: j // 2          # pair index of block j
    side = lambda j: j % 2          # column side within pair tile

    with tile.TileContext(nc) as tc:
        with (
            tc.tile_pool(name="const", bufs=1) as constp,
            tc.tile_pool(name="wrec", bufs=3) as wrecp,
            tc.tile_pool(name="whid", bufs=2) as whidp,
            tc.tile_pool(name="wskip", bufs=4) as wskipp,
            tc.tile_pool(name="curs", bufs=1) as curp,
            tc.tile_pool(name="psum", bufs=1, space="PSUM") as psump,
            tc.tile_pool(name="dram", bufs=3, space="DRAM") as dramp,
        ):
            # ---- persistent inputs -------------------------------------
            hT_sb = constp.tile([128, 64 * B], fp32, name="hT_sb", tag="hT")
            nc.sync.dma_start(out=hT_sb, in_=t_hT[:, :])
            xT_sb = constp.tile([128, 4 * B], fp32, name="xT_sb", tag="xT")
            nc.sync.dma_start(out=xT_sb, in_=t_xT[:, :])
            win_sb = constp.tile([128, 4 * RP], fp32, name="win_sb", tag="win")
            nc.sync.dma_start(out=win_sb, in_=t_win[:, :])
            biasrow_sb = constp.tile([1, 4 * 256], fp32, name="biasrow_sb", tag="br")
            nc.sync.dma_start(out=biasrow_sb, in_=t_biasrow[:, :])
            wout_sb = constp.tile([128, 512], fp32, name="wout_sb", tag="wout")
            nc.sync.dma_start(out=wout_sb, in_=t_wout[:, :])
            boutrow_sb = constp.tile([1, 512], fp32, name="boutrow_sb", tag="bo")
            nc.sync.dma_start(out=boutrow_sb, in_=t_boutrow[:, :])
            ones_sb = constp.tile([1, B], fp32, name="ones_sb", tag="ones")
            nc.sync.dma_start(out=ones_sb, in_=t_ones[:, :])
            ident_sb = constp.tile([B, B], fp32, name="ident_sb", tag="ident")
            nc.sync.dma_start(out=ident_sb, in_=t_ident[:, :])

            psA = [
                psump.tile([64, 256], fp32, name=f"psA{q}", tag=f"psA{q}")
                for q in range(4)
            ]
            curT = [None] * 8

            # bias-init: psA[q] = ones.T @ biasrow[q]  (start=True opens group)
            for q in range(4):
                nc.tensor.matmul(
                    psA[q],
                    lhsT=ones_sb[:, :],
                    rhs=biasrow_sb[:, q * 256:(q + 1) * 256],
                    start=True,
                    stop=False,
                )

            def rec_pair(q):
                """Stream pair q's rec weights (1MB DMA chunks) + matmuls."""
                wide = 256 if q < 3 else 128
                nhalf = 2 if q < 3 else 1
                for h in range(nhalf):
                    rt = wrecp.tile([128, 8192], fp32, name=f"rec{q}h{h}", tag="rec")
                    for i in range(8):
                        nc.sync.dma_start(
                            out=rt[:, i * 1024:(i + 1) * 1024],
                            in_=t_rec[q][:, h * 8192 + i * 1024:
                                         h * 8192 + (i + 1) * 1024],
                        )
                    kn = 8192 // wide
                    for k in range(kn):
                        kg = h * kn + k
                        nc.tensor.matmul(
                            psA[q][:, 0:wide],
                            lhsT=hT_sb[:, kg * B:(kg + 1) * B],
                            rhs=rt[:, k * wide:(k + 1) * wide],
                            start=False,
                            stop=False,
                        )

            def chain_tail(j, last_of_tile):
                """sigmoid -> transpose -> AllGather -> unpack for block j."""
                q, s = qof(j), side(j)
                cp = curp.tile([64, 128], fp32, name=f"cpart{j}", tag="cpart", bufs=2)
                nc.scalar.activation(cp, psA[q][:, s * 128:(s + 1) * 128], AF.Sigmoid)
                pt = psump.tile([128, B], fp32, name=f"pt{j}", tag="pt", bufs=2)
                nc.tensor.transpose(pt, cp, ident_sb[:, :])
                ptsb = curp.tile([128, B], fp32, name=f"ptsb{j}", tag="ptsb", bufs=2)
                nc.vector.tensor_copy(ptsb, pt)
                agin = dramp.tile([128, B], fp32, name=f"agin{j}", tag="agin")
                agout = dramp.tile([W, B], fp32, name=f"agout{j}", tag="agout")
                nc.gpsimd.dma_start(out=agin, in_=ptsb)
                nc.gpsimd.collective_compute(
                    "AllGather",
                    mybir.AluOpType.bypass,
                    replica_groups=rg,
                    ins=[agin.opt()],
                    outs=[agout.opt()],
                )
                dst = curp.tile([128, 8 * B], fp32, name=f"curT{j}", tag=f"curT{j}")
                for kk in range(8):
                    nc.scalar.dma_start(
                        out=dst[:, kk * B:(kk + 1) * B],
                        in_=agout[kk * 128:(kk + 1) * 128, :],
                    )
                curT[j] = dst

            def hid_mms(j):
                """cur_{j-1} @ W_hid[j-1].T into block j's psum columns."""
                q, s = qof(j), side(j)
                ht = whidp.tile([128, 8 * RP], fp32, name=f"hid{j}", tag="hid")
                nc.sync.dma_start(out=ht, in_=t_hid[j - 1, :, :])
                for kk in range(8):
                    nc.tensor.matmul(
                        psA[q][:, s * 128:(s + 1) * 128],
                        lhsT=curT[j - 1][:, kk * B:(kk + 1) * B],
                        rhs=ht[:, kk * RP:(kk + 1) * RP],
                        start=False,
                        stop=(s == 1 and kk == 7),
                    )

            def skip_one(q, c):
                """Pair q's skip contribution sourced from cur_c."""
                a = 2 * q
                wide = 256 if c < a else 128
                st = wskipp.tile(
                    [128, 8 * wide], fp32, name=f"skip{q}_{c}t", tag="skip"
                )
                half = 4 * wide
                for i in range(2):
                    nc.sync.dma_start(
                        out=st[:, i * half:(i + 1) * half],
                        in_=t_skip[(q, c)][:, i * half:(i + 1) * half],
                    )
                off = 0 if c < a else 128
                for kk in range(8):
                    nc.tensor.matmul(
                        psA[q][:, off:off + wide],
                        lhsT=curT[c][:, kk * B:(kk + 1) * B],
                        rhs=st[:, kk * wide:(kk + 1) * wide],
                        start=False,
                        stop=False,
                    )

            # ---- block 0: x @ W_in.T + h @ Wr0.T (+bias) ---------------
            for kk in range(4):
                nc.tensor.matmul(
                    psA[0][:, 0:128],
                    lhsT=xT_sb[:, kk * B:(kk + 1) * B],
                    rhs=win_sb[:, kk * RP:(kk + 1) * RP],
                    start=False,
                    stop=False,
                )
            rec_pair(0)
            chain_tail(0, False)

            rec_pair(1)
            hid_mms(1)
            chain_tail(1, True)
            # lazy skips sourced from cur_0 (feed blocks 2..7)
            skip_one(1, 0)
            skip_one(2, 0)
            skip_one(3, 0)

            rec_pair(2)
            skip_one(1, 1)          # urgent: block 2's last skip source
            hid_mms(2)
            chain_tail(2, False)
            skip_one(2, 1)
            skip_one(3, 1)

            skip_one(1, 2)          # urgent for block 3
            hid_mms(3)
            chain_tail(3, True)
            skip_one(2, 2)
            skip_one(3, 2)

            rec_pair(3)
            skip_one(2, 3)          # urgent for block 4
            hid_mms(4)
            chain_tail(4, False)
            skip_one(3, 3)

            skip_one(2, 4)          # urgent for block 5
            hid_mms(5)
            chain_tail(5, True)
            skip_one(3, 4)

            skip_one(3, 5)          # urgent for block 6 (covers 6 and 7)
            hid_mms(6)
            chain_tail(6, False)

            skip_one(3, 6)          # urgent for block 7
            hid_mms(7)
            # block 7 tail: sigmoid -> transpose -> local out matmul -> RS
            cp7 = curp.tile([64, 128], fp32, name="cpart7", tag="cpart", bufs=2)
            nc.scalar.activation(cp7, psA[3][:, 128:256], AF.Sigmoid)
            pt7 = psump.tile([128, B], fp32, name="pt7", tag="pt", bufs=2)
            nc.tensor.transpose(pt7, cp7, ident_sb[:, :])
            cur7T_sb = curp.tile([128, B], fp32, name="cur7T_sb", tag="c7T")
            nc.vector.tensor_copy(cur7T_sb, pt7)

            pso = psump.tile([64, 512], fp32, name="pso", tag="pso")
            nc.tensor.matmul(
                pso, lhsT=ones_sb[:, :], rhs=boutrow_sb[:, :], start=True, stop=False
            )
            nc.tensor.matmul(
                pso, lhsT=cur7T_sb, rhs=wout_sb[:, :], start=False, stop=True
            )
            out_sb = curp.tile([64, 512], fp32, name="out_sb", tag="out_sb")
            nc.vector.tensor_copy(out_sb, pso)
            rs_in = dramp.tile([64, 512], fp32, name="rs_in", tag="rs_in")
            rs_out = dramp.tile([8, 512], fp32, name="rs_out", tag="rs_out")
            nc.scalar.dma_start(out=rs_in, in_=out_sb)
            nc.gpsimd.collective_compute(
                "ReduceScatter",
                mybir.AluOpType.add,
                replica_groups=rg,
                ins=[rs_in.opt()],
                outs=[rs_out.opt()],
            )
            nc.scalar.dma_start(out=t_out[:, :], in_=rs_out)

    nc.compile()
    return nc


def _get_nc():
    global _BUILT
    if _BUILT is None:
        _BUILT = _build()
    return _BUILT


def make_in_maps(x, hidden_states, W_in, b_in, W_hid, b_hid, W_rec, W_skip,
                 W_out, b_out, mask_hid, mask_rec, mask_skip):
    x = np.asarray(x, np.float32)
    h = np.asarray(hidden_states, np.float32)
    W_in = np.asarray(W_in, np.float32)
    b_in = np.asarray(b_in, np.float32)
    W_out = np.asarray(W_out, np.float32)
    b_out = np.asarray(b_out, np.float32)
    Wh = np.asarray(W_hid, np.float32) * np.asarray(mask_hid, np.float32)
    Wr = np.asarray(W_rec, np.float32) * np.asarray(mask_rec, np.float32)
    Ws = np.asarray(W_skip, np.float32) * np.asarray(mask_skip, np.float32)
    b_hid = np.asarray(b_hid, np.float32)

    hT = _pack(h)
    xT = _pack(x)
    ones = np.ones((1, B), np.float32)
    ident = np.eye(B, dtype=np.float32)
    # every core's partial includes the bias and ReduceScatter sums them
    boutrow = np.ascontiguousarray(b_out[None, :]) / NCORES

    in_maps = []
    for c_ in range(NCORES):
        R = slice(c_ * RP, (c_ + 1) * RP)
        recs = [_pack(Wr[i, R]) for i in range(7)]
        biases = [b_in[R]] + [b_hid[i, R] for i in range(7)]
        biasrow = np.zeros((1, 4 * 256), np.float32)
        for j in range(8):
            biasrow[0, j * 128:(j + 1) * 128] = biases[j]
        m = {
            "hT": hT,
            "xT": xT,
            "win": _pack(W_in[R]),
            "rec0": _interleave(recs[0], recs[1]),
            "rec1": _interleave(recs[2], recs[3]),
            "rec2": _interleave(recs[4], recs[5]),
            "rec3": recs[6],
            "hid": np.stack([_pack(Wh[i, R]) for i in range(7)]),
            "biasrow": biasrow,
            "wout": np.ascontiguousarray(W_out[:, R].T),
            "boutrow": boutrow,
            "ones": ones,
            "ident": ident,
        }
        packs = {j: _pack(Ws[j - 2, R, :j * W]).reshape(128, j * 8, 128)
                 for j in range(2, 8)}
        for q in range(1, 4):
            a = 2 * q
            for c in range(a + 1):
                if c < a:
                    Pa = packs[a][:, c * 8:(c + 1) * 8, :].reshape(128, 8 * 128)
                    Pb = packs[a + 1][:, c * 8:(c + 1) * 8, :].reshape(128, 8 * 128)
                    m[f"skip{q}_{c}"] = _interleave(
                        np.ascontiguousarray(Pa), np.ascontiguousarray(Pb)
                    )
                else:
                    m[f"skip{q}_{c}"] = np.ascontiguousarray(
                        packs[a + 1][:, c * 8:(c + 1) * 8, :].reshape(128, 8 * 128)
                    )
        in_maps.append(m)
    return in_maps


def run(in_maps, **kw):
    from concourse import bass_utils
    nc = _get_nc()
    return bass_utils.run_bass_kernel_spmd(
        nc, in_maps, core_ids=list(range(NCORES)), **kw
    )


def kernel(**inputs):
    in_maps = make_in_maps(**inputs)
    res = run(in_maps)
    return np.ascontiguousarray(
        np.concatenate([res.results[c]["out"] for c in range(NCORES)], axis=0),
        dtype=np.float32,
    )


# revision 20
# speedup vs baseline: 1.2857x; 1.0280x over previous
"""BrainRNN forward pass on 8 TRN2 NeuronCores (Bass/Tile, SPMD).

Strategy (tensor-parallel over output neurons, fp32 exact):
  - Each block's 1024 output neurons are row-sharded 128/core; masks are
    folded into weights on the host (memory-bound: halves HBM traffic).
  - Matmuls run with the (streamed) weights as the MOVING operand and the
    activations as the 64-wide stationary operand: psum[b, m] += sum_k
    actT[k, b] * W.T[k, m].  fp32 moving runs at ~2 cyc/row, so weight
    ingest is ~350-450 GB/s vs ~150 GB/s for weights-stationary.
  - Adjacent blocks are PAIRED into one [64, 256] PSUM accumulator so the
    recurrent matmuls (independent of the sequential chain) run at N=256.
  - Per-block biases enter via a K=1 "ones x biasrow" matmul that also
    opens (start=True) each accumulator.
  - Chain per block: sigmoid (PSUM->SBUF), PE transpose back to
    [128(m), 64(b)], DMA PSUM->DRAM, AllGather over 8 cores, unpack.
  - Output block: each core contracts its own 128 rows of cur7 against
    W_out[:, rows].T (N=512), then one ReduceScatter yields each core an
    8-row batch shard of the [64, 512] output.
"""

import numpy as np

N = 8192
W = 1024
L = 8
B = 64
IN = 512
OUT = 512
NCORES = 8
RP = W // NCORES        # 128 rows per core per block

_BUILT = None


def _pack(A):
    """[M, K] -> [128, (K/128)*M] with packed[p, k*M+m] = A[m, k*128+p].

    Chunk kidx is A[:, kidx*128:(kidx+1)*128].T, i.e. [K=128(part), M(free)].
    """
    M, K = A.shape
    nk = K // 128
    return np.ascontiguousarray(
        A.reshape(M, nk, 128).transpose(2, 1, 0).reshape(128, nk * M)
    )


def _interleave(Pa, Pb):
    """Two packed [128, nk*128] -> [128, nk*256] with per-chunk interleave."""
    nk = Pa.shape[1] // 128
    out = np.empty((128, nk, 2, 128), np.float32)
    out[:, :, 0, :] = Pa.reshape(128, nk, 128)
    out[:, :, 1, :] = Pb.reshape(128, nk, 128)
    return np.ascontiguousarray(out.reshape(128, nk * 256))


def _build():
    import concourse.bass as bass
    import concourse.bacc as bacc
    import concourse.mybir as mybir
    import concourse.tile as tile

    fp32 = mybir.dt.float32
    AF = mybir.ActivationFunctionType

    nc = bacc.Bacc(
        "TRN2",
        target_bir_lowering=False,
        debug=False,
        enable_asserts=False,
        num_devices=NCORES,
    )

    t_hT = nc.dram_tensor("hT", [128, 64 * B], fp32, kind="ExternalInput")
    t_xT = nc.dram_tensor("xT", [128, 4 * B], fp32, kind="ExternalInput")
    t_win = nc.dram_tensor("win", [128, 4 * RP], fp32, kind="ExternalInput")
    t_rec = [
        nc.dram_tensor("rec0", [128, 64 * 256], fp32, kind="ExternalInput"),
        nc.dram_tensor("rec1", [128, 64 * 256], fp32, kind="ExternalInput"),
        nc.dram_tensor("rec2", [128, 64 * 256], fp32, kind="ExternalInput"),
        nc.dram_tensor("rec3", [128, 64 * 128], fp32, kind="ExternalInput"),
    ]
    t_hid = nc.dram_tensor("hid", [7, 128, 8 * RP], fp32, kind="ExternalInput")
    # skip{q}_{c}: pair q=(2q, 2q+1), cur-source block c.  c < 2q: both
    # blocks interleaved [128, 8*256]; c == 2q: later block only [128, 8*128].
    t_skip = {}
    for q in range(1, 4):
        a = 2 * q
        for c in range(a + 1):
            wdt = 8 * 256 if c < a else 8 * 128
            t_skip[(q, c)] = nc.dram_tensor(
                f"skip{q}_{c}", [128, wdt], fp32, kind="ExternalInput"
            )
    t_biasrow = nc.dram_tensor("biasrow", [1, 4 * 256], fp32, kind="ExternalInput")
    t_wout = nc.dram_tensor("wout", [128, 512], fp32, kind="ExternalInput")
    t_boutrow = nc.dram_tensor("boutrow", [1, 512], fp32, kind="ExternalInput")
    t_ones = nc.dram_tensor("ones", [1, B], fp32, kind="ExternalInput")
    t_ident = nc.dram_tensor("ident", [B, B], fp32, kind="ExternalInput")
    t_out = nc.dram_tensor("out", [8, 512], fp32, kind="ExternalOutput")

    rg = [list(range(NCORES))]
    qof = lambda j: j // 2          # pair index of block j
    side = lambda j: j % 2          # column side within pair tile

    with tile.TileContext(nc) as tc:
        with (
            tc.tile_pool(name="const", bufs=1) as constp,
            tc.tile_pool(name="wrec", bufs=3) as wrecp,
            tc.tile_pool(name="whid", bufs=7) as whidp,
            tc.tile_pool(name="wskip", bufs=8) as wskipp,
            tc.tile_pool(name="curs", bufs=1) as curp,
            tc.tile_pool(name="psum", bufs=1, space="PSUM") as psump,
            tc.tile_pool(name="dram", bufs=3, space="DRAM") as dramp,
        ):
            # ---- persistent inputs -------------------------------------
            hT_sb = constp.tile([128, 64 * B], fp32, name="hT_sb", tag="hT")
            nc.sync.dma_start(out=hT_sb, in_=t_hT[:, :])
            xT_sb = constp.tile([128, 4 * B], fp32, name="xT_sb", tag="xT")
            nc.sync.dma_start(out=xT_sb, in_=t_xT[:, :])
            win_sb = constp.tile([128, 4 * RP], fp32, name="win_sb", tag="win")
            nc.sync.dma_start(out=win_sb, in_=t_win[:, :])
            biasrow_sb = constp.tile([1, 4 * 256], fp32, name="biasrow_sb", tag="br")
            nc.sync.dma_start(out=biasrow_sb, in_=t_biasrow[:, :])
            wout_sb = constp.tile([128, 512], fp32, name="wout_sb", tag="wout")
            nc.sync.dma_start(out=wout_sb, in_=t_wout[:, :])
            boutrow_sb = constp.tile([1, 512], fp32, name="boutrow_sb", tag="bo")
            nc.sync.dma_start(out=boutrow_sb, in_=t_boutrow[:, :])
            ones_sb = constp.tile([1, B], fp32, name="ones_sb", tag="ones")
            nc.sync.dma_start(out=ones_sb, in_=t_ones[:, :])
            ident_sb = constp.tile([B, B], fp32, name="ident_sb", tag="ident")
            nc.sync.dma_start(out=ident_sb, in_=t_ident[:, :])

            psA = [
                psump.tile([64, 256], fp32, name=f"psA{q}", tag=f"psA{q}")
                for q in range(4)
            ]
            curT = [None] * 8

            # bias-init: psA[q] = ones.T @ biasrow[q]  (start=True opens group)
            for q in range(4):
                nc.tensor.matmul(
                    psA[q],
                    lhsT=ones_sb[:, :],
                    rhs=biasrow_sb[:, q * 256:(q + 1) * 256],
                    start=True,
                    stop=False,
                )

            def rec_pair(q):
                """Stream pair q's rec weights (2MB tiles, 512KB DMAs) + MMs."""
                wide = 256 if q < 3 else 128
                total = 64 * 256 if q < 3 else 64 * 128
                ntile = total // 4096
                for h in range(ntile):
                    rt = wrecp.tile([128, 4096], fp32, name=f"rec{q}h{h}", tag="rec")
                    for i in range(4):
                        nc.sync.dma_start(
                            out=rt[:, i * 1024:(i + 1) * 1024],
                            in_=t_rec[q][:, h * 4096 + i * 1024:
                                         h * 4096 + (i + 1) * 1024],
                        )
                    kn = 4096 // wide
                    for k in range(kn):
                        kg = h * kn + k
                        nc.tensor.matmul(
                            psA[q][:, 0:wide],
                            lhsT=hT_sb[:, kg * B:(kg + 1) * B],
                            rhs=rt[:, k * wide:(k + 1) * wide],
                            start=False,
                            stop=False,
                        )

            def chain_tail(j, last_of_tile):
                """sigmoid -> transpose -> AllGather -> unpack for block j."""
                q, s = qof(j), side(j)
                cp = curp.tile([64, 128], fp32, name=f"cpart{j}", tag="cpart", bufs=2)
                nc.scalar.activation(cp, psA[q][:, s * 128:(s + 1) * 128], AF.Sigmoid)
                pt = psump.tile([128, B], fp32, name=f"pt{j}", tag="pt", bufs=2)
                nc.tensor.transpose(pt, cp, ident_sb[:, :])
                ptsb = curp.tile([128, B], fp32, name=f"ptsb{j}", tag="ptsb", bufs=2)
                nc.vector.tensor_copy(ptsb, pt)
                agin = dramp.tile([128, B], fp32, name=f"agin{j}", tag="agin")
                agout = dramp.tile([W, B], fp32, name=f"agout{j}", tag="agout")
                nc.gpsimd.dma_start(out=agin, in_=ptsb)
                nc.gpsimd.collective_compute(
                    "AllGather",
                    mybir.AluOpType.bypass,
                    replica_groups=rg,
                    ins=[agin.opt()],
                    outs=[agout.opt()],
                )
                dst = curp.tile([128, 8 * B], fp32, name=f"curT{j}", tag=f"curT{j}")
                for kk in range(2):
                    nc.scalar.dma_start(
                        out=dst[:, kk * 4 * B:(kk + 1) * 4 * B].rearrange(
                            "p (k b) -> p k b", k=4
                        ),
                        in_=agout[kk * 512:(kk + 1) * 512, :].rearrange(
                            "(k p) b -> p k b", p=128
                        ),
                    )
                curT[j] = dst

            def hid_mms(j):
                """cur_{j-1} @ W_hid[j-1].T into block j's psum columns."""
                q, s = qof(j), side(j)
                ht = whidp.tile([128, 8 * RP], fp32, name=f"hid{j}", tag="hid")
                nc.sync.dma_start(out=ht, in_=t_hid[j - 1, :, :])
                for kk in range(8):
                    nc.tensor.matmul(
                        psA[q][:, s * 128:(s + 1) * 128],
                        lhsT=curT[j - 1][:, kk * B:(kk + 1) * B],
                        rhs=ht[:, kk * RP:(kk + 1) * RP],
                        start=False,
                        stop=(s == 1 and kk == 7),
                    )

            skip_tiles = {}

            def skip_tile(q, c):
                """Allocate + DMA pair q's skip weights sourced from cur_c."""
                a = 2 * q
                wide = 256 if c < a else 128
                st = wskipp.tile(
                    [128, 8 * wide], fp32, name=f"skip{q}_{c}t", tag="skip"
                )
                half = 4 * wide
                for i in range(2):
                    nc.sync.dma_start(
                        out=st[:, i * half:(i + 1) * half],
                        in_=t_skip[(q, c)][:, i * half:(i + 1) * half],
                    )
                skip_tiles[(q, c)] = st

            def skip_one(q, c, part=None):
                """MMs for pair q's skip from cur_c.  part: None=all columns,
                'lo'=first block's 128 cols, 'hi'=second block's 128 cols."""
                a = 2 * q
                wide = 256 if c < a else 128
                if (q, c) not in skip_tiles:
                    skip_tile(q, c)
                st = skip_tiles[(q, c)]
                off = 0 if c < a else 128
                rlo, rwide = 0, wide
                if part == "lo" and wide == 256:
                    rlo, rwide = 0, 128
                elif part == "hi" and wide == 256:
                    rlo, rwide = 128, 128
                for kk in range(8):
                    nc.tensor.matmul(
                        psA[q][:, off + rlo:off + rlo + rwide],
                        lhsT=curT[c][:, kk * B:(kk + 1) * B],
                        rhs=st[:, kk * wide + rlo:kk * wide + rlo + rwide],
                        start=False,
                        stop=False,
                    )

            # ---- block 0: x @ W_in.T + h @ Wr0.T (+bias) ---------------
            for kk in range(4):
                nc.tensor.matmul(
                    psA[0][:, 0:128],
                    lhsT=xT_sb[:, kk * B:(kk + 1) * B],
                    rhs=win_sb[:, kk * RP:(kk + 1) * RP],
                    start=False,
                    stop=False,
                )
            rec_pair(0)
            chain_tail(0, False)

            rec_pair(1)
            hid_mms(1)
            chain_tail(1, True)
            # lazy skips sourced from cur_0 (feed blocks 2..7)
            skip_one(1, 0)
            skip_one(2, 0)
            skip_one(3, 0)

            rec_pair(2)
            skip_one(1, 1, part="lo")   # urgent: block 2's last skip source
            hid_mms(2)
            chain_tail(2, False)
            skip_one(1, 1, part="hi")
            skip_one(2, 1)
            skip_one(3, 1)

            skip_one(1, 2)              # urgent for block 3 (128-wide)
            hid_mms(3)
            chain_tail(3, True)
            skip_one(2, 2)
            skip_one(3, 2)

            rec_pair(3)
            skip_one(2, 3, part="lo")   # urgent for block 4
            hid_mms(4)
            chain_tail(4, False)
            skip_one(2, 3, part="hi")
            skip_one(3, 3)

            skip_one(2, 4)              # urgent for block 5 (128-wide)
            hid_mms(5)
            chain_tail(5, True)
            skip_one(3, 4)

            skip_one(3, 5, part="lo")   # urgent for block 6
            hid_mms(6)
            chain_tail(6, False)
            skip_one(3, 5, part="hi")

            skip_one(3, 6)              # urgent for block 7 (128-wide)
            hid_mms(7)
            # block 7 tail: sigmoid -> transpose -> local out matmul -> RS
            cp7 = curp.tile([64, 128], fp32, name="cpart7", tag="cpart", bufs=2)
            nc.scalar.activation(cp7, psA[3][:, 128:256], AF.Sigmoid)
            pt7 = psump.tile([128, B], fp32, name="pt7", tag="pt", bufs=2)
            nc.tensor.transpose(pt7, cp7, ident_sb[:, :])
            cur7T_sb = curp.tile([128, B], fp32, name="cur7T_sb", tag="c7T")
            nc.vector.tensor_copy(cur7T_sb, pt7)

            pso = psump.tile([64, 512], fp32, name="pso", tag="pso")
            nc.tensor.matmul(
                pso, lhsT=ones_sb[:, :], rhs=boutrow_sb[:, :], start=True, stop=False
            )
            nc.tensor.matmul(
                pso, lhsT=cur7T_sb, rhs=wout_sb[:, :], start=False, stop=True
            )
            out_sb = curp.tile([64, 512], fp32, name="out_sb", tag="out_sb")
            nc.vector.tensor_copy(out_sb, pso)
            rs_in = dramp.tile([64, 512], fp32, name="rs_in", tag="rs_in")
            rs_out = dramp.tile([8, 512], fp32, name="rs_out", tag="rs_out")
            nc.scalar.dma_start(out=rs_in, in_=out_sb)
            nc.gpsimd.collective_compute(
                "ReduceScatter",
                mybir.AluOpType.add,
                replica_groups=rg,
                ins=[rs_in.opt()],
                outs=[rs_out.opt()],
            )
            nc.scalar.dma_start(out=t_out[:, :], in_=rs_out)

    nc.compile()
    return nc


def _get_nc():
    global _BUILT
    if _BUILT is None:
        _BUILT = _build()
    return _BUILT


def make_in_maps(x, hidden_states, W_in, b_in, W_hid, b_hid, W_rec, W_skip,
                 W_out, b_out, mask_hid, mask_rec, mask_skip):
    x = np.asarray(x, np.float32)
    h = np.asarray(hidden_states, np.float32)
    W_in = np.asarray(W_in, np.float32)
    b_in = np.asarray(b_in, np.float32)
    W_out = np.asarray(W_out, np.float32)
    b_out = np.asarray(b_out, np.float32)
    Wh = np.asarray(W_hid, np.float32) * np.asarray(mask_hid, np.float32)
    Wr = np.asarray(W_rec, np.float32) * np.asarray(mask_rec, np.float32)
    Ws = np.asarray(W_skip, np.float32) * np.asarray(mask_skip, np.float32)
    b_hid = np.asarray(b_hid, np.float32)

    hT = _pack(h)
    xT = _pack(x)
    ones = np.ones((1, B), np.float32)
    ident = np.eye(B, dtype=np.float32)
    # every core's partial includes the bias and ReduceScatter sums them
    boutrow = np.ascontiguousarray(b_out[None, :]) / NCORES

    in_maps = []
    for c_ in range(NCORES):
        R = slice(c_ * RP, (c_ + 1) * RP)
        recs = [_pack(Wr[i, R]) for i in range(7)]
        biases = [b_in[R]] + [b_hid[i, R] for i in range(7)]
        biasrow = np.zeros((1, 4 * 256), np.float32)
        for j in range(8):
            biasrow[0, j * 128:(j + 1) * 128] = biases[j]
        m = {
            "hT": hT,
            "xT": xT,
            "win": _pack(W_in[R]),
            "rec0": _interleave(recs[0], recs[1]),
            "rec1": _interleave(recs[2], recs[3]),
            "rec2": _interleave(recs[4], recs[5]),
            "rec3": recs[6],
            "hid": np.stack([_pack(Wh[i, R]) for i in range(7)]),
            "biasrow": biasrow,
            "wout": np.ascontiguousarray(W_out[:, R].T),
            "boutrow": boutrow,
            "ones": ones,
            "ident": ident,
        }
        packs = {j: _pack(Ws[j - 2, R, :j * W]).reshape(128, j * 8, 128)
                 for j in range(2, 8)}
        for q in range(1, 4):
            a = 2 * q
            for c in range(a + 1):
                if c < a:
                    Pa = packs[a][:, c * 8:(c + 1) * 8, :].reshape(128, 8 * 128)
                    Pb = packs[a + 1][:, c * 8:(c + 1) * 8, :].reshape(128, 8 * 128)
                    m[f"skip{q}_{c}"] = _interleave(
                        np.ascontiguousarray(Pa), np.ascontiguousarray(Pb)
                    )
                else:
                    m[f"skip{q}_{c}"] = np.ascontiguousarray(
                        packs[a + 1][:, c * 8:(c + 1) * 8, :].reshape(128, 8 * 128)
                    )
        in_maps.append(m)
    return in_maps


def run(in_maps, **kw):
    from concourse import bass_utils
    nc = _get_nc()
    return bass_utils.run_bass_kernel_spmd(
        nc, in_maps, core_ids=list(range(NCORES)), **kw
    )


def kernel(**inputs):
    in_maps = make_in_maps(**inputs)
    res = run(in_maps)
    return np.ascontiguousarray(
        np.concatenate([res.results[c]["out"] for c in range(NCORES)], axis=0),
        dtype=np.float32,
    )
